# revision 24
# baseline (speedup 1.0000x reference)
"""Single-head attention (InterModalAttention) Bass kernel for 8 TRN2 cores.

Sharding: batch (4) x seq-half (2) -> 8 cores. Core (2b+h) projects Q/K/V
for its OWN 1024 rows of batch b. K and V are exchanged within the pair
(2b, 2b+1) via pairwise AllGather of the own half (K in 2 pieces, V in
2 pieces); each core recovers the peer half with the rank-symmetric
blend peer = row0 + row1 - own (DVE for K -> fp8, gpsimd for V -> f16),
computed in place over the gathered rows. AllGather is ~2x faster than
AllReduce on the CC cores (no reduce pass), and the blend costs only one
extra elementwise pass in otherwise-idle engine windows. The own half
never leaves SBUF, and every core processes keys in the order
[own 1024, peer 1024]; attention is permutation-invariant over keys, so
own-key score/output tiles have no collective dependency.

Precision (numpy sim rel-err 1.58e-2 vs the 2e-2 gate; HW matches sim):
  - fp16 for projections and attn@V (same PE rate as bf16, 8x lower
    quantization error); fp32 accumulation in PSUM; f16 exchange.
  - scores matmul in fp8-e4m3 perf_mode=DoubleRow: contracts 256/MM,
    halving score matmul count. The [P, et, cols] layout keeps et-pairs
    adjacent so DoubleRow's [Ki, 2, free] AP falls out directly.

Bias-via-matmul: the Q bias and the output bias are folded into the
PSUM accumulations as K=1 matmuls (bq16.T @ ones_row, rs16.T @ bv16),
so the Q epilogue is a pure ACT fp8 convert (no DVE on the scores-start
critical path) and the output epilogue is ACT-scale -> DMA only.

DMA plan: many ~128-256KB pieces alternating the two hardware trigger
engines (parallel rings; single big DMAs serialize on one ring at
~80GB/s). Exchange row-loads, rowsum bounces and output stores all sit
on sync in completion order; every per-engine FIFO is monotone in
data-ready time so no trigger head-of-line blocking.
"""
import sys
import numpy as np

for p in ("/opt/trn_rl_repo",):
    if p not in sys.path:
        sys.path.insert(0, p)

B, S, D = 4, 2048, 1024
NQ = 1024          # queries (and own keys) per core
NCORES = 8
P = 128
INV_SQRT_D = 1.0 / 32.0
PAIRS = [[0, 1], [2, 3], [4, 5], [6, 7]]

_CACHE = {}


def build_nc():
    from contextlib import ExitStack
    import concourse.mybir as mybir
    import concourse.tile as tile
    from concourse import bacc

    F32 = mybir.dt.float32
    F16 = mybir.dt.float16
    F8 = mybir.dt.float8e4
    AF = mybir.ActivationFunctionType
    DR = mybir.MatmulPerfMode.DoubleRow
    SUB = mybir.AluOpType.subtract
    ADD = mybir.AluOpType.add

    nc = bacc.Bacc("TRN2", debug=False, num_devices=NCORES)

    ET = D // P            # 8 e-tiles
    DT = D // P            # 8 d-tiles
    HC = NQ // 512         # 2 s-chunks over own half
    SB = S // P            # 16 j-tiles (per-core order: 0-7 own, 8-15 peer)
    HB = NQ // P           # 8 j-tiles (own half)
    IG = NQ // 512         # 2 i-chunks
    EC = D // 512          # 2 e-chunks
    ETH = ET // 2          # 4 et-pairs for DoubleRow

    # inputs pre-transposed on host into SBUF layouts
    x2 = nc.dram_tensor("x2", (HC, P, DT, 512), F16, kind="ExternalInput")
    wq2 = nc.dram_tensor("wq2", (P, DT, D), F16, kind="ExternalInput")
    wk2 = nc.dram_tensor("wk2", (P, DT, D), F16, kind="ExternalInput")
    wv2 = nc.dram_tensor("wv2", (P, DT, D), F16, kind="ExternalInput")
    bq = nc.dram_tensor("bq", (D,), F32, kind="ExternalInput")
    bk = nc.dram_tensor("bk", (D,), F32, kind="ExternalInput")
    bv = nc.dram_tensor("bv", (D,), F32, kind="ExternalInput")
    out = nc.dram_tensor("out", (NQ, D), F32, kind="ExternalOutput")

    with tile.TileContext(nc) as tc, ExitStack() as ctx:
        consts = ctx.enter_context(tc.tile_pool(name="consts", bufs=1))

        # resident tensors
        kqv = ctx.enter_context(tc.tile_pool(name="kqv", bufs=1))
        kT8 = kqv.tile([P, ET, S], F8)       # [d-part, e-tile, key] own|peer
        qT8 = kqv.tile([P, ET, NQ], F8)      # [d-part, e-tile, i]
        vN = kqv.tile([P, SB, D], F16)       # [j-part, j-tile, e] own|peer
        kf16 = kqv.tile([P, ET, NQ], F16)    # own K, f16 (bounce + blend)
        krows = kqv.tile([P, ET, NQ], F16)   # gathered K rows, one piece
        vrows = kqv.tile([P, 4, 2 * D], F16)  # gathered V rows, one piece

        # DRAM buffers for the pairwise K/V AllGather (2 pieces each)
        ccd = ctx.enter_context(tc.tile_pool(name="ccd", bufs=1, space="DRAM"))
        kb_in = [ccd.tile([P, ET, 512], F16, tag=f"kbi{c}", name=f"kbi{c}")
                 for c in range(HC)]
        kb_out = [ccd.tile([2, P, ET, 512], F16, tag=f"kbo{c}", name=f"kbo{c}")
                  for c in range(HC)]
        vb_in = [ccd.tile([P, 4, D], F16, tag=f"vbi{c}", name=f"vbi{c}")
                 for c in range(HC)]
        vb_out = [ccd.tile([2, P, 4, D], F16, tag=f"vbo{c}", name=f"vbo{c}")
                  for c in range(HC)]

        # pp spans K1/V/Q projections AND scores; closed before outps.
        pp_stack = ExitStack()

        # ---- Phase 1: projections over own half, single pass over x ----
        with tc.tile_pool(name="w", bufs=1) as wp, \
             tc.tile_pool(name="xc", bufs=2) as xcp:
            wk_sb = wp.tile([P, DT, D], F16)
            wq_sb = wp.tile([P, DT, D], F16)
            wv_sb = wp.tile([P, DT, D], F16)
            xc = []
            for hc in range(HC):
                xc.append(xcp.tile([P, DT, 512], F16, tag="xc", name=f"xc{hc}"))

            # biases + consts first (tiny). bq/bv as f16 rows for the
            # bias-via-matmul trick; bk as per-partition f32 for DVE adds.
            bk_sb = consts.tile([P, ET], F32)
            nc.scalar.dma_start(bk_sb[:], bk[:].rearrange("(t p) -> p t", p=P))
            bq16 = consts.tile([1, D], F16)
            nc.gpsimd.dma_start(bq16[:], bq[:].rearrange("(one d) -> one d", one=1))
            bv16 = consts.tile([1, D], F16)
            nc.gpsimd.dma_start(bv16[:], bv[:].rearrange("(one d) -> one d", one=1))
            ones_row = consts.tile([1, 512], F16)
            nc.vector.memset(ones_row[:], 1.0)
            onesb = consts.tile([P, 1], F32)
            nc.vector.memset(onesb[:], 1.0)
            rs16s = [consts.tile([1, 512], F16, tag=f"rs16_{g}", name=f"rs16_{g}")
                     for g in range(IG)]

            # big loads: consumption order (wk,x0) -> x1 -> wv -> wq as
            # per-dt pieces alternating the two hw queues (parallel rings)
            _eng = [nc.sync, nc.scalar]
            _dmac = [0]
            def dma(out_ap, in_ap):
                e = _eng[_dmac[0] % len(_eng)]
                _dmac[0] += 1
                e.dma_start(out_ap, in_ap)

            for dt in range(DT):
                dma(wk_sb[:, dt, :], wk2[:, dt, :])
                dma(xc[0][:, dt, :], x2[0, :, dt, :])
            for dt in range(DT):
                dma(xc[1][:, dt, :], x2[1, :, dt, :])
            for dt in range(DT):
                dma(wv_sb[:, dt, :], wv2[:, dt, :])
            for dt in range(DT):
                dma(wq_sb[:, dt, :], wq2[:, dt, :])

            # K chunk 0: dt-outer/et-inner over 8 PSUM banks so the PE
            # starts as soon as the first pieces land.
            with tc.tile_pool(name="p8", bufs=8, space="PSUM") as p8:
                psk0 = [p8.tile([P, 512], F32, tag="p8", name=f"psk0_{et}")
                        for et in range(ET)]
                for dt in range(DT):
                    for et in range(ET):
                        nc.tensor.matmul(psk0[et][:],
                                         wk_sb[:, dt, et * P:(et + 1) * P],
                                         xc[0][:, dt, :], start=(dt == 0),
                                         stop=(dt == DT - 1))
                for et in range(ET):
                    nc.vector.tensor_scalar_add(kf16[:, et, 0:512],
                                                psk0[et][:], bk_sb[:, et:et + 1])
                    nc.scalar.activation(kT8[:, et, 0:512], kf16[:, et, 0:512],
                                         AF.Copy)
            # K chunk 0 -> bounce (scalar; its load queue drains first)
            nc.scalar.dma_start(kb_in[0][:], kf16[:, :, 0:512])
            nc.gpsimd.collective_compute(
                "AllGather", mybir.AluOpType.bypass, replica_groups=PAIRS,
                ins=[kb_in[0][:].opt()], outs=[kb_out[0][:].opt()])

            pp = pp_stack.enter_context(
                tc.tile_pool(name="pp", bufs=4, space="PSUM"))
            # K chunk 1
            for et in range(ET):
                psk = pp.tile([P, 512], F32, tag="pp")
                for dt in range(DT):
                    nc.tensor.matmul(psk[:], wk_sb[:, dt, et * P:(et + 1) * P],
                                     xc[1][:, dt, :], start=(dt == 0),
                                     stop=(dt == DT - 1))
                nc.vector.tensor_scalar_add(kf16[:, et, 512:1024],
                                            psk[:], bk_sb[:, et:et + 1])
                nc.scalar.activation(kT8[:, et, 512:1024], kf16[:, et, 512:1024],
                                     AF.Copy)
            # K chunk 1 -> bounce + AllGather piece 1
            nc.sync.dma_start(kb_in[1][:], kf16[:, :, 512:1024])
            nc.gpsimd.collective_compute(
                "AllGather", mybir.AluOpType.bypass, replica_groups=PAIRS,
                ins=[kb_in[1][:].opt()], outs=[kb_out[1][:].opt()])

            # V projection (own half) -> vN j-tiles 0..7 (copies on ACT);
            # bounce + AllGather per 4-tile half
            for hc in range(HC):
                for sb_i in range(4):
                    jg = hc * 4 + sb_i
                    for ec in range(EC):
                        psv = pp.tile([P, 512], F32, tag="pp")
                        for dt in range(DT):
                            nc.tensor.matmul(psv[:],
                                             xc[hc][:, dt, sb_i * P:(sb_i + 1) * P],
                                             wv_sb[:, dt, ec * 512:(ec + 1) * 512],
                                             start=(dt == 0), stop=(dt == DT - 1))
                        nc.scalar.activation(
                            vN[:, jg, ec * 512:(ec + 1) * 512], psv[:], AF.Copy)
                nc.scalar.dma_start(vb_in[hc][:], vN[:, hc * 4:(hc + 1) * 4, :])
                nc.gpsimd.collective_compute(
                    "AllGather", mybir.AluOpType.bypass, replica_groups=PAIRS,
                    ins=[vb_in[hc][:].opt()], outs=[vb_out[hc][:].opt()])

            # K rows in (sync; loads there drain by ~30us) + DVE blends:
            # krows[0:512] += krows[512:]; kT8_peer = krows[0:512] - own.
            # Piece 1 reuses the krows buffer after piece 0's blends.
            for c in range(HC):
                for r in range(2):
                    nc.sync.dma_start(krows[:, :, r * 512:(r + 1) * 512],
                                      kb_out[c][r])
                for et in range(ET):
                    nc.vector.tensor_tensor(krows[:, et, 0:512],
                                            krows[:, et, 0:512],
                                            krows[:, et, 512:1024], op=ADD)
                for et in range(ET):
                    nc.vector.tensor_tensor(
                        kT8[:, et, NQ + c * 512:NQ + (c + 1) * 512],
                        krows[:, et, 0:512],
                        kf16[:, et, c * 512:(c + 1) * 512], op=SUB)

            # Q projection -> fp8; bias folded in as a K=1 matmul so the
            # epilogue is a pure ACT convert
            for hc in range(HC):
                for et in range(ET):
                    psq = pp.tile([P, 512], F32, tag="pp")
                    for dt in range(DT):
                        nc.tensor.matmul(psq[:],
                                         wq_sb[:, dt, et * P:(et + 1) * P],
                                         xc[hc][:, dt, :], start=(dt == 0),
                                         stop=False)
                    nc.tensor.matmul(psq[:], bq16[:, et * P:(et + 1) * P],
                                     ones_row[:], start=False, stop=True)
                    nc.scalar.activation(
                        qT8[:, et, hc * 512:(hc + 1) * 512], psq[:], AF.Copy)

            # V rows in (sync) + gpsimd blends into vN peer tiles
            for c in range(HC):
                for r in range(2):
                    nc.sync.dma_start(vrows[:, :, r * D:(r + 1) * D],
                                      vb_out[c][r])
                for jg4 in range(4):
                    jg = c * 4 + jg4
                    nc.gpsimd.tensor_tensor(vrows[:, jg4, 0:D],
                                            vrows[:, jg4, 0:D],
                                            vrows[:, jg4, D:2 * D], op=ADD)
                    nc.gpsimd.tensor_tensor(vN[:, HB + jg, :],
                                            vrows[:, jg4, 0:D],
                                            vN[:, jg, :], op=SUB)

        # ---- Phase 2: scores (fp8 DoubleRow) then output matmuls ----
        with tc.tile_pool(name="attn", bufs=1) as attnp, \
             tc.tile_pool(name="epi2", bufs=2) as epi2p, \
             tc.tile_pool(name="rsdram", bufs=2, space="DRAM") as rsdram, \
             tc.tile_pool(name="epi", bufs=2) as epip:
            attnTs = [attnp.tile([P, SB, 512], F16, tag=f"attnT{g}", name=f"attnT{g}")
                      for g in range(IG)]
            accs = [epip.tile([P, 512], F32, tag="acc", name=f"acc{g}")
                    for g in range(IG)]
            invss = [epi2p.tile([P, 4], F32, tag="invs", name=f"invs{g}")
                     for g in range(IG)]

            def scores_tile(g, jt):
                attnT = attnTs[g]
                sc_ps = pp.tile([P, 512], F32, tag="pp")
                for t in range(ETH):
                    nc.tensor.matmul(
                        sc_ps[:],
                        kT8[:, 2 * t:2 * t + 2, jt * P:(jt + 1) * P],
                        qT8[:, 2 * t:2 * t + 2, g * 512:(g + 1) * 512],
                        start=(t == 0), stop=(t == ETH - 1),
                        perf_mode=DR)
                nc.scalar.activation(attnT[:, jt, :], sc_ps[:], AF.Exp,
                                     scale=INV_SQRT_D)
                if jt == 0:
                    nc.vector.tensor_copy(accs[g][:], attnT[:, 0, :])
                else:
                    nc.vector.tensor_add(accs[g][:], accs[g][:], attnT[:, jt, :])

            def rowsum(g, psum_pool):
                # ones-matmul partition sum -> DRAM-bounce transpose -> 1/x
                rs = psum_pool.tile([1, 512], F32, tag="rs", name=f"rs{g}",
                                    bufs=1)
                nc.tensor.matmul(rs[:], onesb[:], accs[g][:], start=True,
                                 stop=True)
                rs_sb = epip.tile([1, 512], F32, tag="rs_sb")
                nc.vector.tensor_copy(rs_sb[:], rs[:])
                nc.scalar.activation(rs16s[g][:], rs_sb[:], AF.Copy)
                rs_d = rsdram.tile([1, 512], F32, tag="rs_d")
                nc.sync.dma_start(rs_d[:], rs_sb[:])
                rsT = epip.tile([P, 4], F32, tag="rsT")
                nc.sync.dma_start(
                    rsT[:], rs_d[:].rearrange("one (b p) -> p (one b)", p=P))
                nc.vector.reciprocal(invss[g][:], rsT[:])

            def out_block(g, ib, outps, nsl, last=False):
                # out_psum = sum_j attn_j @ v_j + rs.T @ bv  (K=1 matmul),
                # so out = psum * (1/rs) needs no separate bias add.
                attnT = attnTs[g]
                i0 = ib * P
                ops = [outps.tile([P, 512], F32, tag=f"outps{ec}",
                                  name=f"ops{g}_{ib}_{ec}") for ec in range(EC)]
                for jt in range(SB):
                    for ec in range(EC):
                        nc.tensor.matmul(ops[ec][:],
                                         attnT[:, jt, i0:i0 + P],
                                         vN[:, jt, ec * 512:(ec + 1) * 512],
                                         start=(jt == 0), stop=False)
                for ec in range(EC):
                    nc.tensor.matmul(ops[ec][:],
                                     rs16s[g][:, i0:i0 + P],
                                     bv16[:, ec * 512:(ec + 1) * 512],
                                     start=False, stop=True)
                out_sb = epi2p.tile([P, D], F32, tag="out_sb")
                r0 = g * 512 + i0
                w = D // nsl
                for sl_i in range(nsl):
                    sl = slice(sl_i * w, (sl_i + 1) * w)
                    ec = (sl_i * w) // 512
                    psl = slice(sl_i * w - ec * 512, (sl_i + 1) * w - ec * 512)
                    nc.scalar.activation(out_sb[:, sl], ops[ec][:, psl],
                                         AF.Copy, scale=invss[g][:, ib:ib + 1])
                    # last block drains via two queues in parallel
                    eng = nc.gpsimd if (last and sl_i < nsl // 2) else nc.sync
                    eng.dma_start(out[r0:r0 + P, sl], out_sb[:, sl])

            for jt in range(SB):
                scores_tile(0, jt)
            scores_tile(1, 0)
            scores_tile(1, 1)
            rowsum(0, pp)       # acc0 is complete; PE is busy in g1 tiles
            for jt in range(2, SB):
                scores_tile(1, jt)
            pp_stack.close()
            with tc.tile_pool(name="outps", bufs=3, space="PSUM") as outps:
                out_block(0, 0, outps, 2)
                rowsum(1, outps)    # acc1 complete; PE busy in output block
                out_block(0, 1, outps, 2)
                out_block(0, 2, outps, 2)
                out_block(0, 3, outps, 2)
                out_block(1, 0, outps, 2)
                out_block(1, 1, outps, 2)
                out_block(1, 2, outps, 4)
                out_block(1, 3, outps, 4, last=True)

    nc.compile()
    return nc


def make_in_maps(x, Wq, bq, Wk, bk, Wv, bv):
    F16 = np.float16
    x = np.asarray(x, np.float32)

    def wprep(W):
        # [P, DT, D] with [p, t, d] = W.T[t*128+p, d]
        wT = np.asarray(W, np.float32).T.astype(F16)
        return np.ascontiguousarray(wT.reshape(8, P, D).transpose(1, 0, 2))

    wq2, wk2, wv2 = wprep(Wq), wprep(Wk), wprep(Wv)
    bq = np.ascontiguousarray(np.asarray(bq, np.float32))
    bk = np.ascontiguousarray(np.asarray(bk, np.float32))
    bv = np.ascontiguousarray(np.asarray(bv, np.float32))
    in_maps = []
    for c in range(NCORES):
        b, h = c // 2, c % 2
        xT = x[b][h * NQ:(h + 1) * NQ].T.astype(F16)     # [D, NQ]
        # [HC, P, DT, 512] with [hc, p, t, j] = xT[t*128+p, hc*512+j]
        x2 = xT.reshape(8, P, 2, 512).transpose(2, 1, 0, 3)
        in_maps.append({
            "x2": np.ascontiguousarray(x2),
            "wq2": wq2, "wk2": wk2, "wv2": wv2,
            "bq": bq, "bk": bk, "bv": bv,
        })
    return in_maps


def get_nc():
    if "nc" not in _CACHE:
        _CACHE["nc"] = build_nc()
    return _CACHE["nc"]


def kernel(x, Wq, bq, Wk, bk, Wv, bv):
    from concourse.bass_utils import run_bass_kernel_spmd
    nc = get_nc()
    in_maps = make_in_maps(x, Wq, bq, Wk, bk, Wv, bv)
    res = run_bass_kernel_spmd(nc, in_maps, core_ids=list(range(NCORES)))
    out = np.empty((B, S, D), np.float32)
    for c in range(NCORES):
        b, h = c // 2, c % 2
        out[b, h * NQ:(h + 1) * NQ] = res.results[c]["out"]
    return out


# revision 30
# speedup vs baseline: 1.1580x; 1.1580x over previous
"""Single-head attention (InterModalAttention) Bass kernel for 8 TRN2 cores.

Sharding: batch (4) x seq-half (2) -> 8 cores. Core (2b+h) projects Q/K/V
for its OWN 1024 rows of batch b. K and V are exchanged within the pair
(2b, 2b+1) via pairwise AllGather of the own half (K in 2 pieces, V in
2 pieces); each core recovers the peer half with the rank-symmetric
blend peer = row0 + row1 - own (DVE for K -> fp8, gpsimd for V -> f16),
computed in place over the gathered rows. AllGather is ~2x faster than
AllReduce on the CC cores (no reduce pass), and the blend costs only one
extra elementwise pass in otherwise-idle engine windows. The own half
never leaves SBUF, and every core processes keys in the order
[own 1024, peer 1024]; attention is permutation-invariant over keys, so
own-key score/output tiles have no collective dependency.

Precision (numpy sim rel-err 1.58e-2 vs the 2e-2 gate; HW matches sim):
  - fp16 for projections and attn@V (same PE rate as bf16, 8x lower
    quantization error); fp32 accumulation in PSUM; f16 exchange.
  - scores matmul in fp8-e4m3 perf_mode=DoubleRow: contracts 256/MM,
    halving score matmul count. The [P, et, cols] layout keeps et-pairs
    adjacent so DoubleRow's [Ki, 2, free] AP falls out directly.

Bias-via-matmul: the Q bias and the output bias are folded into the
PSUM accumulations as K=1 matmuls (bq16.T @ ones_row, rs16.T @ bv16),
so the Q epilogue is a pure ACT fp8 convert (no DVE on the scores-start
critical path) and the output epilogue is ACT-scale -> DMA only.

DMA plan: many ~128-256KB pieces alternating the two hardware trigger
engines (parallel rings; single big DMAs serialize on one ring at
~80GB/s). Exchange row-loads, rowsum bounces and output stores all sit
on sync in completion order; every per-engine FIFO is monotone in
data-ready time so no trigger head-of-line blocking.
"""
import sys
import numpy as np

for p in ("/opt/trn_rl_repo",):
    if p not in sys.path:
        sys.path.insert(0, p)

B, S, D = 4, 2048, 1024
NQ = 1024          # queries (and own keys) per core
NCORES = 8
P = 128
INV_SQRT_D = 1.0 / 32.0
PAIRS = [[0, 1], [2, 3], [4, 5], [6, 7]]

_CACHE = {}


def build_nc():
    from contextlib import ExitStack
    import concourse.mybir as mybir
    import concourse.tile as tile
    from concourse import bacc

    F32 = mybir.dt.float32
    F16 = mybir.dt.float16
    F8 = mybir.dt.float8e4
    AF = mybir.ActivationFunctionType
    DR = mybir.MatmulPerfMode.DoubleRow
    SUB = mybir.AluOpType.subtract
    ADD = mybir.AluOpType.add

    nc = bacc.Bacc("TRN2", debug=False, num_devices=NCORES)

    ET = D // P            # 8 e-tiles
    DT = D // P            # 8 d-tiles
    HC = NQ // 512         # 2 s-chunks over own half
    SB = S // P            # 16 j-tiles (per-core order: 0-7 own, 8-15 peer)
    HB = NQ // P           # 8 j-tiles (own half)
    IG = NQ // 512         # 2 i-chunks
    EC = D // 512          # 2 e-chunks
    ETH = ET // 2          # 4 et-pairs for DoubleRow

    # inputs pre-transposed on host into SBUF layouts
    x2 = nc.dram_tensor("x2", (HC, P, DT, 512), F16, kind="ExternalInput")
    wq2 = nc.dram_tensor("wq2", (P, DT, D), F16, kind="ExternalInput")
    wk2 = nc.dram_tensor("wk2", (P, DT, D), F16, kind="ExternalInput")
    wv2 = nc.dram_tensor("wv2", (P, DT, D), F16, kind="ExternalInput")
    bq = nc.dram_tensor("bq", (D,), F32, kind="ExternalInput")
    bk = nc.dram_tensor("bk", (D,), F32, kind="ExternalInput")
    bv = nc.dram_tensor("bv", (D,), F32, kind="ExternalInput")
    out = nc.dram_tensor("out", (NQ, D), F32, kind="ExternalOutput")

    with tile.TileContext(nc) as tc, ExitStack() as ctx:
        consts = ctx.enter_context(tc.tile_pool(name="consts", bufs=1))

        # resident tensors
        kqv = ctx.enter_context(tc.tile_pool(name="kqv", bufs=1))
        kT8 = kqv.tile([P, ET, S], F8)       # [d-part, e-tile, key] own|peer
        qT8 = kqv.tile([P, ET, NQ], F8)      # [d-part, e-tile, i]
        vN = kqv.tile([P, SB, D], F16)       # [j-part, j-tile, e] own|peer
        krows = kqv.tile([P, ET, NQ], F8)    # gathered K rows, one piece
        ktmp = kqv.tile([P, ET, 512], F16)   # r0+r1 (exact) for the K blend
        vrows = kqv.tile([P, 4, 2 * D], F16)  # gathered V rows, one piece

        # DRAM buffers for the pairwise K/V AllGather (2 pieces each).
        # K crosses as fp8: AllGather moves bits unchanged, and the blend
        # peer = (r0 + r1) - own cancels own exactly, so the peer fp8 is
        # bit-exact -- half the bytes of f16 at zero precision cost.
        ccd = ctx.enter_context(tc.tile_pool(name="ccd", bufs=1, space="DRAM"))
        kb_in = [ccd.tile([P, ET, 512], F8, tag=f"kbi{c}", name=f"kbi{c}")
                 for c in range(HC)]
        kb_out = [ccd.tile([2, P, ET, 512], F8, tag=f"kbo{c}", name=f"kbo{c}")
                  for c in range(HC)]
        vb_in = [ccd.tile([P, 4, D], F16, tag=f"vbi{c}", name=f"vbi{c}")
                 for c in range(HC)]
        vb_out = [ccd.tile([2, P, 4, D], F16, tag=f"vbo{c}", name=f"vbo{c}")
                  for c in range(HC)]

        # pp spans K1/V/Q projections AND scores; closed before outps.
        pp_stack = ExitStack()

        # ---- Phase 1: projections over own half, single pass over x ----
        with tc.tile_pool(name="w", bufs=1) as wp, \
             tc.tile_pool(name="xc", bufs=2) as xcp:
            wk_sb = wp.tile([P, DT, D], F16)
            wq_sb = wp.tile([P, DT, D], F16)
            wv_sb = wp.tile([P, DT, D], F16)
            xc = []
            for hc in range(HC):
                xc.append(xcp.tile([P, DT, 512], F16, tag="xc", name=f"xc{hc}"))

            # biases + consts first (tiny). bq/bv as f16 rows for the
            # bias-via-matmul trick; bk as per-partition f32 for DVE adds.
            bk_sb = consts.tile([P, ET], F32)
            nc.scalar.dma_start(bk_sb[:], bk[:].rearrange("(t p) -> p t", p=P))
            bq16 = consts.tile([1, D], F16)
            nc.gpsimd.dma_start(bq16[:], bq[:].rearrange("(one d) -> one d", one=1))
            bv16 = consts.tile([1, D], F16)
            nc.gpsimd.dma_start(bv16[:], bv[:].rearrange("(one d) -> one d", one=1))
            ones_row = consts.tile([1, 512], F16)
            nc.vector.memset(ones_row[:], 1.0)
            onesb = consts.tile([P, 1], F32)
            nc.vector.memset(onesb[:], 1.0)
            rs16s = [consts.tile([1, 512], F16, tag=f"rs16_{g}", name=f"rs16_{g}")
                     for g in range(IG)]

            # big loads: consumption order (wk,x0) -> x1 -> wv -> wq as
            # per-dt pieces alternating the two hw queues (parallel rings)
            _eng = [nc.sync, nc.scalar]
            _dmac = [0]
            def dma(out_ap, in_ap):
                e = _eng[_dmac[0] % len(_eng)]
                _dmac[0] += 1
                e.dma_start(out_ap, in_ap)

            for dt in range(DT):
                dma(wk_sb[:, dt, :], wk2[:, dt, :])
                dma(xc[0][:, dt, :], x2[0, :, dt, :])
            for dt in range(DT):
                dma(xc[1][:, dt, :], x2[1, :, dt, :])
            for dt in range(DT):
                dma(wv_sb[:, dt, :], wv2[:, dt, :])
            for dt in range(DT):
                dma(wq_sb[:, dt, :], wq2[:, dt, :])

            # K chunk 0: dt-outer/et-inner over 8 PSUM banks so the PE
            # starts as soon as the first pieces land.
            with tc.tile_pool(name="p8", bufs=8, space="PSUM") as p8:
                psk0 = [p8.tile([P, 512], F32, tag="p8", name=f"psk0_{et}")
                        for et in range(ET)]
                for dt in range(DT):
                    for et in range(ET):
                        nc.tensor.matmul(psk0[et][:],
                                         wk_sb[:, dt, et * P:(et + 1) * P],
                                         xc[0][:, dt, :], start=(dt == 0),
                                         stop=(dt == DT - 1))
                for et in range(ET):
                    nc.vector.tensor_scalar_add(kT8[:, et, 0:512],
                                                psk0[et][:], bk_sb[:, et:et + 1])
            # K chunk 0 -> bounce (scalar; its load queue drains first)
            nc.scalar.dma_start(kb_in[0][:], kT8[:, :, 0:512])
            nc.gpsimd.collective_compute(
                "AllGather", mybir.AluOpType.bypass, replica_groups=PAIRS,
                ins=[kb_in[0][:].opt()], outs=[kb_out[0][:].opt()])

            pp = pp_stack.enter_context(
                tc.tile_pool(name="pp", bufs=4, space="PSUM"))
            # K chunk 1
            for et in range(ET):
                psk = pp.tile([P, 512], F32, tag="pp")
                for dt in range(DT):
                    nc.tensor.matmul(psk[:], wk_sb[:, dt, et * P:(et + 1) * P],
                                     xc[1][:, dt, :], start=(dt == 0),
                                     stop=(dt == DT - 1))
                nc.vector.tensor_scalar_add(kT8[:, et, 512:1024],
                                            psk[:], bk_sb[:, et:et + 1])
            # K chunk 1 -> bounce + AllGather piece 1
            nc.sync.dma_start(kb_in[1][:], kT8[:, :, 512:1024])
            nc.gpsimd.collective_compute(
                "AllGather", mybir.AluOpType.bypass, replica_groups=PAIRS,
                ins=[kb_in[1][:].opt()], outs=[kb_out[1][:].opt()])

            # V projection (own half) -> vN j-tiles 0..7 (copies on ACT);
            # bounce + AllGather per 4-tile half
            for hc in range(HC):
                for sb_i in range(4):
                    jg = hc * 4 + sb_i
                    for ec in range(EC):
                        psv = pp.tile([P, 512], F32, tag="pp")
                        for dt in range(DT):
                            nc.tensor.matmul(psv[:],
                                             xc[hc][:, dt, sb_i * P:(sb_i + 1) * P],
                                             wv_sb[:, dt, ec * 512:(ec + 1) * 512],
                                             start=(dt == 0), stop=(dt == DT - 1))
                        nc.scalar.activation(
                            vN[:, jg, ec * 512:(ec + 1) * 512], psv[:], AF.Copy)
                nc.scalar.dma_start(vb_in[hc][:], vN[:, hc * 4:(hc + 1) * 4, :])
                nc.gpsimd.collective_compute(
                    "AllGather", mybir.AluOpType.bypass, replica_groups=PAIRS,
                    ins=[vb_in[hc][:].opt()], outs=[vb_out[hc][:].opt()])

            # K rows in (sync; loads there drain by ~30us) + gpsimd blends:
            # ktmp = r0 + r1 (exact in f16); kT8_peer = ktmp - own (fp8,
            # bit-exact peer). Piece 1 reuses krows after piece 0's blends.
            for c in range(HC):
                for r in range(2):
                    nc.sync.dma_start(krows[:, :, r * 512:(r + 1) * 512],
                                      kb_out[c][r])
                for et in range(ET):
                    nc.gpsimd.tensor_tensor(ktmp[:, et, :],
                                            krows[:, et, 0:512],
                                            krows[:, et, 512:1024], op=ADD)
                for et in range(ET):
                    nc.gpsimd.tensor_tensor(
                        kT8[:, et, NQ + c * 512:NQ + (c + 1) * 512],
                        ktmp[:, et, :],
                        kT8[:, et, c * 512:(c + 1) * 512], op=SUB)

            # Q projection -> fp8; bias folded in as a K=1 matmul so the
            # epilogue is a pure ACT convert
            for hc in range(HC):
                for et in range(ET):
                    psq = pp.tile([P, 512], F32, tag="pp")
                    for dt in range(DT):
                        nc.tensor.matmul(psq[:],
                                         wq_sb[:, dt, et * P:(et + 1) * P],
                                         xc[hc][:, dt, :], start=(dt == 0),
                                         stop=False)
                    nc.tensor.matmul(psq[:], bq16[:, et * P:(et + 1) * P],
                                     ones_row[:], start=False, stop=True)
                    nc.scalar.activation(
                        qT8[:, et, hc * 512:(hc + 1) * 512], psq[:], AF.Copy)

            # V rows in (sync) + gpsimd blends into vN peer tiles
            for c in range(HC):
                for r in range(2):
                    nc.sync.dma_start(vrows[:, :, r * D:(r + 1) * D],
                                      vb_out[c][r])
                for jg4 in range(4):
                    jg = c * 4 + jg4
                    nc.gpsimd.tensor_tensor(vrows[:, jg4, 0:D],
                                            vrows[:, jg4, 0:D],
                                            vrows[:, jg4, D:2 * D], op=ADD)
                    nc.gpsimd.tensor_tensor(vN[:, HB + jg, :],
                                            vrows[:, jg4, 0:D],
                                            vN[:, jg, :], op=SUB)

        # ---- Phase 2: scores (fp8 DoubleRow) then output matmuls ----
        with tc.tile_pool(name="attn", bufs=1) as attnp, \
             tc.tile_pool(name="epi2", bufs=2) as epi2p, \
             tc.tile_pool(name="rsdram", bufs=2, space="DRAM") as rsdram, \
             tc.tile_pool(name="epi", bufs=2) as epip:
            attnTs = [attnp.tile([P, SB, 512], F16, tag=f"attnT{g}", name=f"attnT{g}")
                      for g in range(IG)]
            accs = [epip.tile([P, 512], F32, tag="acc", name=f"acc{g}")
                    for g in range(IG)]
            invss = [epi2p.tile([P, 4], F32, tag="invs", name=f"invs{g}")
                     for g in range(IG)]

            def scores_tile(g, jt):
                attnT = attnTs[g]
                sc_ps = pp.tile([P, 512], F32, tag="pp")
                for t in range(ETH):
                    nc.tensor.matmul(
                        sc_ps[:],
                        kT8[:, 2 * t:2 * t + 2, jt * P:(jt + 1) * P],
                        qT8[:, 2 * t:2 * t + 2, g * 512:(g + 1) * 512],
                        start=(t == 0), stop=(t == ETH - 1),
                        perf_mode=DR)
                nc.scalar.activation(attnT[:, jt, :], sc_ps[:], AF.Exp,
                                     scale=INV_SQRT_D)
                if jt == 0:
                    nc.vector.tensor_copy(accs[g][:], attnT[:, 0, :])
                else:
                    nc.vector.tensor_add(accs[g][:], accs[g][:], attnT[:, jt, :])

            def rowsum(g, psum_pool):
                # ones-matmul partition sum -> DRAM-bounce transpose -> 1/x
                rs = psum_pool.tile([1, 512], F32, tag="rs", name=f"rs{g}",
                                    bufs=1)
                nc.tensor.matmul(rs[:], onesb[:], accs[g][:], start=True,
                                 stop=True)
                rs_sb = epip.tile([1, 512], F32, tag="rs_sb")
                nc.vector.tensor_copy(rs_sb[:], rs[:])
                nc.scalar.activation(rs16s[g][:], rs_sb[:], AF.Copy)
                rs_d = rsdram.tile([1, 512], F32, tag="rs_d")
                nc.sync.dma_start(rs_d[:], rs_sb[:])
                rsT = epip.tile([P, 4], F32, tag="rsT")
                nc.sync.dma_start(
                    rsT[:], rs_d[:].rearrange("one (b p) -> p (one b)", p=P))
                nc.vector.reciprocal(invss[g][:], rsT[:])

            def out_blockA(g, ib, outps):
                # own-key half of the accumulation (no collective dep)
                attnT = attnTs[g]
                i0 = ib * P
                ops = [outps.tile([P, 512], F32, tag=f"outps{ec}",
                                  name=f"ops{g}_{ib}_{ec}") for ec in range(EC)]
                for jt in range(HB):
                    for ec in range(EC):
                        nc.tensor.matmul(ops[ec][:],
                                         attnT[:, jt, i0:i0 + P],
                                         vN[:, jt, ec * 512:(ec + 1) * 512],
                                         start=(jt == 0), stop=False)
                return ops

            def out_blockB(g, ib, ops, nsl, last=False):
                # peer-key half + rs.T @ bv bias matmul + epilogue.
                # out_psum ends as sum_j attn_j @ v_j + rs.T @ bv, so
                # out = psum * (1/rs) needs no separate bias add.
                attnT = attnTs[g]
                i0 = ib * P
                for jt in range(HB, SB):
                    for ec in range(EC):
                        nc.tensor.matmul(ops[ec][:],
                                         attnT[:, jt, i0:i0 + P],
                                         vN[:, jt, ec * 512:(ec + 1) * 512],
                                         start=False, stop=False)
                for ec in range(EC):
                    nc.tensor.matmul(ops[ec][:],
                                     rs16s[g][:, i0:i0 + P],
                                     bv16[:, ec * 512:(ec + 1) * 512],
                                     start=False, stop=True)
                out_sb = epi2p.tile([P, D], F32, tag="out_sb")
                r0 = g * 512 + i0
                w = D // nsl
                for sl_i in range(nsl):
                    sl = slice(sl_i * w, (sl_i + 1) * w)
                    ec = (sl_i * w) // 512
                    psl = slice(sl_i * w - ec * 512, (sl_i + 1) * w - ec * 512)
                    nc.scalar.activation(out_sb[:, sl], ops[ec][:, psl],
                                         AF.Copy, scale=invss[g][:, ib:ib + 1])
                    # last block drains via two queues in parallel
                    eng = nc.gpsimd if (last and sl_i < nsl // 2) else nc.sync
                    eng.dma_start(out[r0:r0 + P, sl], out_sb[:, sl])

            for jt in range(SB):
                scores_tile(0, jt)
            scores_tile(1, 0)
            scores_tile(1, 1)
            rowsum(0, pp)       # acc0 is complete; PE is busy in g1 tiles
            for jt in range(2, SB):
                scores_tile(1, jt)
            pp_stack.close()
            # Output phase, A/B split: own-key passes (A) for 3 blocks run
            # before the first peer-key pass (B), buying the V exchange
            # ~10us of extra cover. 3 blocks in flight (6 banks + rs).
            with tc.tile_pool(name="outps", bufs=3, space="PSUM") as outps:
                blocks = [(0, 0), (0, 1), (0, 2), (0, 3),
                          (1, 0), (1, 1), (1, 2), (1, 3)]
                ops_of = {}
                for idx in range(3):
                    ops_of[idx] = out_blockA(*blocks[idx], outps)
                for idx in range(3, 8):
                    bidx = idx - 3
                    g, ib = blocks[bidx]
                    out_blockB(g, ib, ops_of.pop(bidx), 2)
                    if bidx == 0:
                        rowsum(1, outps)   # acc1 done; PE busy in blocks
                    ops_of[idx] = out_blockA(*blocks[idx], outps)
                for bidx in range(5, 8):
                    g, ib = blocks[bidx]
                    nsl = 4 if bidx >= 6 else 2
                    out_blockB(g, ib, ops_of.pop(bidx), nsl, last=(bidx == 7))

    nc.compile()
    return nc


def make_in_maps(x, Wq, bq, Wk, bk, Wv, bv):
    F16 = np.float16
    x = np.asarray(x, np.float32)

    def wprep(W):
        # [P, DT, D] with [p, t, d] = W.T[t*128+p, d]
        wT = np.asarray(W, np.float32).T.astype(F16)
        return np.ascontiguousarray(wT.reshape(8, P, D).transpose(1, 0, 2))

    wq2, wk2, wv2 = wprep(Wq), wprep(Wk), wprep(Wv)
    bq = np.ascontiguousarray(np.asarray(bq, np.float32))
    bk = np.ascontiguousarray(np.asarray(bk, np.float32))
    bv = np.ascontiguousarray(np.asarray(bv, np.float32))
    in_maps = []
    for c in range(NCORES):
        b, h = c // 2, c % 2
        xT = x[b][h * NQ:(h + 1) * NQ].T.astype(F16)     # [D, NQ]
        # [HC, P, DT, 512] with [hc, p, t, j] = xT[t*128+p, hc*512+j]
        x2 = xT.reshape(8, P, 2, 512).transpose(2, 1, 0, 3)
        in_maps.append({
            "x2": np.ascontiguousarray(x2),
            "wq2": wq2, "wk2": wk2, "wv2": wv2,
            "bq": bq, "bk": bk, "bv": bv,
        })
    return in_maps


def get_nc():
    if "nc" not in _CACHE:
        _CACHE["nc"] = build_nc()
    return _CACHE["nc"]


def kernel(x, Wq, bq, Wk, bk, Wv, bv):
    from concourse.bass_utils import run_bass_kernel_spmd
    nc = get_nc()
    in_maps = make_in_maps(x, Wq, bq, Wk, bk, Wv, bv)
    res = run_bass_kernel_spmd(nc, in_maps, core_ids=list(range(NCORES)))
    out = np.empty((B, S, D), np.float32)
    for c in range(NCORES):
        b, h = c // 2, c % 2
        out[b, h * NQ:(h + 1) * NQ] = res.results[c]["out"]
    return out
